# revision 1
# baseline (speedup 1.0000x reference)
"""Trainium2 Bass kernel for nn_Attention_57543971831928.

Dense pre-LN self-attention block:
  t = x.reshape(B,C,N).T ; t_norm = LN(t)
  qkv = t_norm @ W1.T + b1 ; attention (6 heads, d=64) ; o = att_out @ W2.T + b2
  out = (o + t_norm).T.reshape(B,C,H,W)

Sharding: data-parallel over batch B=8, one batch element per NeuronCore,
no collectives.  Inside each core everything is computed in the
"transposed" [c, n] / [j, n] layout so every matmul contraction sits on
the partition axis:

  - LayerNorm stats need per-n reductions over c, so x is PE-transposed
    to [n, c] tiles, normalized there (bn_stats/bn_aggr), and the bf16
    result transposed back to [c, n].
  - q^T/k^T [j, n] = W1^T-stationary matmuls; V [n, j] uses t_norm^T
    blocks as the stationary operand (saves transposing V later).
  - Scores are computed transposed, S^T[n_j, n_i] = k_h^T.T @ q_h^T,
    with two heads packed into the 128x128 PE array as 64-row tiles
    (K=d=64 each).  exp() runs on ScalarE straight out of PSUM for both
    heads in one instruction (softmax denominator is deferred).
  - PV uses E^T blocks as the stationary operand and V_h augmented with
    a ones column, so O'[n_i, 0:64] and the softmax denominator
    r[n_i] = O'[n_i, 64] come out of the same accumulation.  A
    reciprocal + scalar-mul normalizes afterwards.

The softmax exp is the roofline: B*h*N^2/8 = 31.85M elements/core
through ScalarE at 128 lanes * 1.2 GHz ~= 210 us; everything else is
overlapped against it.
"""

import sys

sys.path.insert(0, "/opt/trn_rl_repo")

import numpy as np
import orjson

import concourse.bass as bass
import concourse.mybir as mybir
import concourse.tile as tile
from concourse.masks import make_identity

# ---------------------------------------------------------------------------
# Workaround: the walrus build in this container only accepts a single
# sync-wait per instruction; Tile emits instructions waiting on several
# semaphores.  Split extra waits onto preceding same-engine NoOps at BIR
# serialization time.
# ---------------------------------------------------------------------------
_SYNC_WAIT_LIMIT = 1


def _fix_inst_list(insts):
    out = []
    for inst in insts:
        si = inst.get("sync_info")
        ow = (si or {}).get("on_wait") or []
        if si is not None and len(ow) > _SYNC_WAIT_LIMIT:
            keep = ow[-_SYNC_WAIT_LIMIT:]
            extras = ow[:-_SYNC_WAIT_LIMIT]
            for i, w in enumerate(extras):
                out.append(
                    {
                        "debug": inst.get("debug", 0),
                        "engine": inst["engine"],
                        "ins": [],
                        "outs": [],
                        "name": f"{inst['name']}.w{i}",
                        "opcode": "NoOp",
                        "sync_info": {"on_update": [], "on_wait": [w]},
                    }
                )
            si["on_wait"] = keep
        out.append(inst)
    return out


def _walk_fix(obj):
    if isinstance(obj, dict):
        for k, v in obj.items():
            if k == "instructions" and isinstance(v, list):
                obj[k] = _fix_inst_list(v)
                for inst in obj[k]:
                    _walk_fix(inst)
            else:
                _walk_fix(v)
    elif isinstance(obj, list):
        for v in obj:
            _walk_fix(v)


if not getattr(bass.Bass, "_ant_wait_split_patched", False):
    _orig_to_json_bytes = bass.Bass.to_json_bytes

    def _patched_to_json_bytes(self):
        m = orjson.loads(_orig_to_json_bytes(self))
        _walk_fix(m)
        return orjson.dumps(m)

    bass.Bass.to_json_bytes = _patched_to_json_bytes
    bass.Bass._ant_wait_split_patched = True

# ---------------------------------------------------------------------------
# Problem constants (hardcoded per task instructions)
# ---------------------------------------------------------------------------
B = 8
C = 384
H = W = 48
N = H * W          # 2304
NH = 6             # heads
D = C // NH        # 64
J3 = 3 * C         # 1152
P = 128
CT = C // P        # 3 c-tiles
NT = N // P        # 18 n-tiles
LN_EPS = 1e-5

F32 = mybir.dt.float32
BF16 = mybir.dt.bfloat16



def build_nc(reps: int = 1) -> bass.Bass:
    nc = bass.Bass()
    x_ext = nc.dram_tensor("x", [C, N], F32, kind="ExternalInput")
    w1_ext = nc.dram_tensor("W1", [J3, C], F32, kind="ExternalInput")
    b1_ext = nc.dram_tensor("b1", [J3], F32, kind="ExternalInput")
    w2_ext = nc.dram_tensor("W2", [C, C], F32, kind="ExternalInput")
    b2_ext = nc.dram_tensor("b2", [C], F32, kind="ExternalInput")
    out_ext = nc.dram_tensor("out", [C, N], F32, kind="ExternalOutput")

    with tile.TileContext(nc) as tc:
        for _ in range(reps):
            _build_body(nc, tc, x_ext, w1_ext, b1_ext, w2_ext, b2_ext, out_ext)
    return nc


def _build_body(nc, tc, x_ext, w1_ext, b1_ext, w2_ext, b2_ext, out_ext):
    from contextlib import ExitStack

    with ExitStack() as ctx:
        singles = ctx.enter_context(tc.tile_pool(name="singles", bufs=1))

        # ---- constants -----------------------------------------------------
        ident_f32 = singles.tile([P, P], F32)
        make_identity(nc, ident_f32)
        ident_bf = singles.tile([P, P], BF16)
        make_identity(nc, ident_bf)
        eps_sb = singles.tile([P, 1], F32)
        nc.vector.memset(eps_sb, LN_EPS)

        # b1 laid out partition-major per j-tile: b1_sb[p, jt] = b1[jt*128+p]
        b1_ap = b1_ext[:]
        b2_ap = b2_ext[:]
        b1_sb = singles.tile([P, J3 // P], F32)
        nc.sync.dma_start(
            out=b1_sb,
            in_=bass.AP(tensor=b1_ap.tensor, offset=b1_ap.offset,
                        ap=[[1, P], [P, J3 // P]]),
        )
        b2_sb = singles.tile([P, C // P], F32)
        nc.sync.dma_start(
            out=b2_sb,
            in_=bass.AP(tensor=b2_ap.tensor, offset=b2_ap.offset,
                        ap=[[1, P], [P, C // P]]),
        )
        # b1 slice for V, single row (broadcast via K=1 matmul later)
        b1v_f32 = singles.tile([1, C], F32)
        nc.sync.dma_start(
            out=b1v_f32,
            in_=bass.AP(tensor=b1_ap.tensor, offset=b1_ap.offset + 2 * C,
                        ap=[[1, 1], [1, C]]),
        )
        b1v_sb = singles.tile([1, C], BF16)
        nc.vector.tensor_copy(b1v_sb, b1v_f32)

        # ---- W1^T / W2^T (bf16, [c, j] layout) ----------------------------
        w1t_sb = [singles.tile([P, J3], BF16, name=f"w1t{i}") for i in range(CT)]
        w2t_sb = [singles.tile([P, C], BF16, name=f"w2t{i}") for i in range(CT)]

        with (
            tc.tile_pool(name="wrows", bufs=3) as wrows,
            tc.tile_pool(name="wpsum", bufs=4, space="PSUM") as wpsum,
        ):
            for jt in range(J3 // P):
                wr = wrows.tile([P, C], F32, tag="wrow")
                (nc.sync if jt % 2 == 0 else nc.scalar).dma_start(
                    out=wr, in_=w1_ext[jt * P:(jt + 1) * P, :])
                for ct in range(CT):
                    ps = wpsum.tile([P, P], F32, tag="wT")
                    nc.tensor.transpose(ps, wr[:, ct * P:(ct + 1) * P], ident_f32)
                    nc.any.tensor_copy(w1t_sb[ct][:, jt * P:(jt + 1) * P], ps)
            for rt in range(CT):
                wr = wrows.tile([P, C], F32, tag="wrow")
                nc.sync.dma_start(out=wr, in_=w2_ext[rt * P:(rt + 1) * P, :])
                for ct in range(CT):
                    ps = wpsum.tile([P, P], F32, tag="wT")
                    nc.tensor.transpose(ps, wr[:, ct * P:(ct + 1) * P], ident_f32)
                    nc.any.tensor_copy(w2t_sb[ct][:, rt * P:(rt + 1) * P], ps)

        # ---- persistent activations ---------------------------------------
        tn_cn = [singles.tile([P, N], BF16, name=f"tn_cn{i}") for i in range(CT)]
        qkT = [singles.tile([P, N], BF16, name=f"qkT{i}") for i in range(2 * C // P)]
        v_sb = [singles.tile([P, NH, D + 1], BF16, name=f"v{i}") for i in range(NT)]
        o_nc = [singles.tile([P, NH, D], BF16, name=f"o_nc{i}") for i in range(NT)]

        # ---- LayerNorm -----------------------------------------------------
        with (
            tc.tile_pool(name="xin", bufs=1) as xin,
            tc.tile_pool(name="ln", bufs=4) as ln,
            tc.tile_pool(name="lnps", bufs=5, space="PSUM") as lnps,
            tc.tile_pool(name="tn_nc_pool", bufs=4) as tn_nc_pool,
            tc.tile_pool(name="tps", bufs=3, space="PSUM") as tps,
        ):
            x_sb = [xin.tile([P, N], F32, name=f"x_sb{i}") for i in range(CT)]
            dma_engines = [nc.sync, nc.scalar, nc.gpsimd]
            for ct in range(CT):
                dma_engines[ct].dma_start(out=x_sb[ct],
                                          in_=x_ext[ct * P:(ct + 1) * P, :])

            for nt in range(NT):
                pt = lnps.tile([P, C], F32, tag="xt")      # t tile [n, c]
                for ct in range(CT):
                    nc.tensor.transpose(
                        pt[:, ct * P:(ct + 1) * P],
                        x_sb[ct][:, nt * P:(nt + 1) * P],
                        ident_f32,
                    )
                stats = ln.tile([P, nc.vector.BN_STATS_DIM], F32, tag="stats")
                nc.vector.bn_stats(out=stats, in_=pt)
                mv = ln.tile([P, nc.vector.BN_AGGR_DIM], F32, tag="mv")
                nc.vector.bn_aggr(out=mv, in_=stats)
                rstd = ln.tile([P, 1], F32, tag="rstd")
                nc.scalar.activation(
                    out=rstd, in_=mv[:, 1:2],
                    func=mybir.ActivationFunctionType.Sqrt,
                    bias=eps_sb, scale=1.0, alpha=0.0,
                )
                nc.vector.reciprocal(out=rstd, in_=rstd)
                tn = tn_nc_pool.tile([P, C], BF16, tag="tn_nc")
                nc.vector.tensor_scalar(
                    out=tn, in0=pt,
                    scalar1=mv[:, 0:1], scalar2=rstd,
                    op0=mybir.AluOpType.subtract, op1=mybir.AluOpType.mult,
                )
                # transpose t_norm back to [c, n]
                for ct in range(CT):
                    pc = tps.tile([P, P], BF16, tag="tnT")
                    nc.tensor.transpose(pc, tn[:, ct * P:(ct + 1) * P], ident_bf)
                    nc.scalar.copy(tn_cn[ct][:, nt * P:(nt + 1) * P], pc)

        # ---- QKV -----------------------------------------------------------
        N_SUBS = [(s, min(512, N - s)) for s in range(0, N, 512)]
        with tc.tile_pool(name="qkps", bufs=4, space="PSUM") as qkps:
            ones_row = singles.tile([1, P], BF16, name="ones_row")
            nc.vector.memset(ones_row, 1.0)
            for nt in range(NT):  # V in [n, j] layout, with ones column
                ps = qkps.tile([P, C], F32, tag="v")
                for ct in range(CT):
                    nc.tensor.matmul(
                        ps,
                        tn_cn[ct][:, nt * P:(nt + 1) * P],
                        w1t_sb[ct][:, 2 * C:3 * C],
                        start=(ct == 0), stop=False,
                    )
                # + b1v broadcast to every row via a K=1 ones-row matmul
                nc.tensor.matmul(ps, ones_row, b1v_sb[0:1, :],
                                 start=False, stop=True)
                nc.vector.memset(v_sb[nt][:, :, D:D + 1], 1.0)
                nc.vector.tensor_copy(
                    v_sb[nt].rearrange("p h d -> p (h d)")[:, : NH * (D + 1)]
                    .rearrange("p (h d) -> p h d", h=NH)[:, :, 0:D],
                    ps.rearrange("p (h d) -> p h d", h=NH),
                )

            for jt in (0, 3, 1, 4, 2, 5):  # q^T/k^T j-tiles, head-pair-0 tiles first
                for s0, sl in N_SUBS:
                    ps = qkps.tile([P, 512], F32, tag="qk")
                    for ct in range(CT):
                        nc.tensor.matmul(
                            ps[:, :sl],
                            w1t_sb[ct][:, jt * P:(jt + 1) * P],
                            tn_cn[ct][:, s0:s0 + sl],
                            start=(ct == 0), stop=(ct == CT - 1),
                        )
                    nc.scalar.activation(
                        out=qkT[jt][:, s0:s0 + sl], in_=ps[:, :sl],
                        func=mybir.ActivationFunctionType.Identity,
                        bias=b1_sb[:, jt:jt + 1], scale=1.0,
                    )

        # ---- attention + projection, n_i-chunk outer ----------------------
        # Per 512-wide n_i chunk: all 3 head pairs run S^T -> exp -> PV,
        # normalize into o_cn[:, chunk]; then the output projection +
        # residual for that chunk issues immediately (overlaps the next
        # chunk's attention on PE/DVE while ScalarE stays exp-bound).
        o_cn = [singles.tile([P, N], BF16, name=f"o_cn{i}") for i in range(CT)]
        CHUNKS = [(s, min(512, N - s)) for s in range(0, N, 512)]
        with (
            tc.tile_pool(name="et", bufs=4) as etp,
            tc.tile_pool(name="sps", bufs=2, space="PSUM") as sps,
            tc.tile_pool(name="ops", bufs=1, space="PSUM") as ops,
            tc.tile_pool(name="rbps", bufs=1, space="PSUM") as rbps,
            tc.tile_pool(name="pps", bufs=1, space="PSUM") as pps,
            tc.tile_pool(name="nrm", bufs=4) as nrm,
            tc.tile_pool(name="outp", bufs=3) as outp,
        ):
            ones_bf = singles.tile([1, D], BF16, name="ones_bf")
            nc.vector.memset(ones_bf, 1.0)
            # Flat software-pipelined schedule over (chunk, head-pair, njt):
            # the S^T pair for step k+1 issues BEFORE the exp-dependent PV of
            # step k, so the PE fills the exp latency and ScalarE runs
            # back-to-back (HW-probed: 1647 -> 735 ns/step).
            steps = [(ci, hp, njt)
                     for ci in range(len(CHUNKS))
                     for hp in range(NH // 2)
                     for njt in range(NT)]

            def s_pair(ci, hp, njt):
                c0, cl = CHUNKS[ci]
                ps_s = sps.tile([P, 2, 512], F32, tag="S", name=f"s{ci}_{hp}_{njt}")
                for h2 in range(2):
                    nc.tensor.matmul(
                        ps_s[:, h2, 0:cl],
                        qkT[NH // 2 + hp][h2 * D:(h2 + 1) * D,
                                          njt * P:(njt + 1) * P],
                        qkT[hp][h2 * D:(h2 + 1) * D, c0:c0 + cl],
                        start=True, stop=True,
                    )
                return ps_s

            po = None
            s_pend = s_pair(*steps[0])
            pending = []   # deferred normalize/proj closures, flushed one pair later

            def make_finish(ci, hp, ou_pair):
                c0, cl = CHUNKS[ci]

                def finish():
                    # normalize: recip row -> bf16 -> K=1 ones-matmul broadcast
                    # across 64 psum partitions -> elementwise multiply.
                    for h2 in range(2):
                        ou = ou_pair[h2]
                        rrow = nrm.tile([1, 512], F32, tag="rrow",
                                        name=f"rr{ci}_{hp}_{h2}")
                        nc.vector.reciprocal(out=rrow[:, 0:cl],
                                             in_=ou[D:D + 1, 0:cl])
                        rrow_bf = nrm.tile([1, 512], BF16, tag="rrow_bf",
                                           name=f"rrb{ci}_{hp}_{h2}")
                        nc.vector.tensor_copy(rrow_bf[:, 0:cl], rrow[:, 0:cl])
                        rbp = rbps.tile([P, 512], F32, tag="rb",
                                        name=f"rbp{ci}_{hp}_{h2}")
                        nc.tensor.matmul(
                            rbp[h2 * D:(h2 + 1) * D, 0:cl],
                            ones_bf,
                            rrow_bf[:, 0:cl],
                            start=True, stop=True,
                        )
                        nc.vector.tensor_tensor(
                            o_cn[hp][h2 * D:(h2 + 1) * D, c0:c0 + cl],
                            ou[0:D, 0:cl],
                            rbp[h2 * D:(h2 + 1) * D, 0:cl],
                            mybir.AluOpType.mult,
                        )
                    if hp == NH // 2 - 1:
                        # output projection + residual for this chunk
                        for rt in range(CT):
                            ps = pps.tile([P, 512], F32, tag="proj",
                                          name=f"pj{ci}_{rt}")
                            for ct in range(CT):
                                nc.tensor.matmul(
                                    ps[:, :cl],
                                    w2t_sb[ct][:, rt * P:(rt + 1) * P],
                                    o_cn[ct][:, c0:c0 + cl],
                                    start=(ct == 0), stop=(ct == CT - 1),
                                )
                            out_sb = outp.tile([P, 512], F32, tag="out",
                                               name=f"ot{ci}_{rt}")
                            nc.vector.scalar_tensor_tensor(
                                out=out_sb[:, :cl],
                                in0=ps[:, :cl],
                                scalar=b2_sb[:, rt:rt + 1],
                                in1=tn_cn[rt][:, c0:c0 + cl],
                                op0=mybir.AluOpType.add,
                                op1=mybir.AluOpType.add,
                            )
                            nc.sync.dma_start(
                                out=out_ext[rt * P:(rt + 1) * P, c0:c0 + cl],
                                in_=out_sb[:, :cl])

                return finish

            for k, (ci, hp, njt) in enumerate(steps):
                c0, cl = CHUNKS[ci]
                if njt == 0:
                    po = [ops.tile([P, 512], F32, tag=f"O{i}",
                                   name=f"po{ci}_{hp}_{i}") for i in range(2)]
                et = etp.tile([P, 2, 512], BF16, tag="ET")
                nc.scalar.activation(
                    out=et[:, :, 0:cl], in_=s_pend[:, :, 0:cl],
                    func=mybir.ActivationFunctionType.Exp,
                    scale=0.125,
                )
                if k + 1 < len(steps):
                    s_pend = s_pair(*steps[k + 1])
                for h2 in range(2):
                    nc.tensor.matmul(
                        po[h2][0:D + 1, 0:cl],
                        v_sb[njt][:, 2 * hp + h2, :],
                        et[:, h2, 0:cl],
                        start=(njt == 0), stop=(njt == NT - 1),
                    )
                if njt == 4 and pending:
                    pending.pop(0)()
                if njt == NT - 1:
                    # Only stage O' out of PSUM now (frees po quickly); the
                    # PE-touching normalize/proj is deferred one head-pair so
                    # its DVE dependency chain completes off the critical path.
                    ou_pair = []
                    for h2 in range(2):
                        ou = nrm.tile([P, 512], F32, tag=f"ou{h2}",
                                      name=f"ou{ci}_{hp}_{h2}")
                        nc.vector.tensor_copy(ou[0:D + 1, 0:cl],
                                              po[h2][0:D + 1, 0:cl])
                        ou_pair.append(ou)
                    pending.append(make_finish(ci, hp, ou_pair))
            while pending:
                pending.pop(0)()


# ---------------------------------------------------------------------------
# host-side entry point
# ---------------------------------------------------------------------------
_NC_CACHE = {}


def _get_nc(reps: int = 1):
    if reps not in _NC_CACHE:
        _NC_CACHE[reps] = build_nc(reps)
    return _NC_CACHE[reps]


def kernel(x, W1, b1, W2, b2):
    from concourse.bass_utils import run_bass_kernel_spmd

    nc = _get_nc()
    x = np.ascontiguousarray(x, dtype=np.float32)
    in_maps = [
        {
            "x": x[b].reshape(C, N),
            "W1": np.ascontiguousarray(W1, dtype=np.float32),
            "b1": np.ascontiguousarray(b1, dtype=np.float32),
            "W2": np.ascontiguousarray(W2, dtype=np.float32),
            "b2": np.ascontiguousarray(b2, dtype=np.float32),
        }
        for b in range(B)
    ]
    res = run_bass_kernel_spmd(nc, in_maps, core_ids=list(range(B)))
    out = np.stack([res.results[b]["out"] for b in range(B)], axis=0)
    return out.reshape(B, C, H, W).astype(np.float32)



# revision 33
# speedup vs baseline: 1.1548x; 1.1548x over previous
"""Trainium2 Bass kernel for nn_Attention_57543971831928.

Dense pre-LN self-attention block:
  t = x.reshape(B,C,N).T ; t_norm = LN(t)
  qkv = t_norm @ W1.T + b1 ; attention (6 heads, d=64) ; o = att_out @ W2.T + b2
  out = (o + t_norm).T.reshape(B,C,H,W)

Sharding: data-parallel over batch B=8, one batch element per NeuronCore,
no collectives.  Inside each core everything is computed in the
"transposed" [c, n] / [j, n] layout so every matmul contraction sits on
the partition axis:

  - LayerNorm stats need per-n reductions over c, so x is PE-transposed
    to [n, c] tiles, normalized there (bn_stats/bn_aggr), and the bf16
    result transposed back to [c, n].
  - q^T/k^T [j, n] = W1^T-stationary matmuls; V [n, j] uses t_norm^T
    blocks as the stationary operand (saves transposing V later).
  - Scores are computed transposed, S^T[n_j, n_i] = k_h^T.T @ q_h^T,
    with two heads packed into the 128x128 PE array as 64-row tiles
    (K=d=64 each).  exp() runs on ScalarE straight out of PSUM for both
    heads in one instruction (softmax denominator is deferred).
  - PV uses E^T blocks as the stationary operand and V_h augmented with
    a ones column, so O'[n_i, 0:64] and the softmax denominator
    r[n_i] = O'[n_i, 64] come out of the same accumulation.  A
    reciprocal + scalar-mul normalizes afterwards.

The softmax exp is the roofline: B*h*N^2/8 = 31.85M elements/core
through ScalarE at 128 lanes * 1.2 GHz ~= 210 us; everything else is
overlapped against it.
"""

import sys

sys.path.insert(0, "/opt/trn_rl_repo")

import numpy as np
import orjson

import concourse.bass as bass
import concourse.mybir as mybir
import concourse.tile as tile
from concourse.masks import make_identity

# ---------------------------------------------------------------------------
# Workaround: the walrus build in this container only accepts a single
# sync-wait per instruction; Tile emits instructions waiting on several
# semaphores.  Split extra waits onto preceding same-engine NoOps at BIR
# serialization time.
# ---------------------------------------------------------------------------
_SYNC_WAIT_LIMIT = 1


def _fix_inst_list(insts):
    out = []
    for inst in insts:
        si = inst.get("sync_info")
        ow = (si or {}).get("on_wait") or []
        if si is not None and len(ow) > _SYNC_WAIT_LIMIT:
            keep = ow[-_SYNC_WAIT_LIMIT:]
            extras = ow[:-_SYNC_WAIT_LIMIT]
            for i, w in enumerate(extras):
                out.append(
                    {
                        "debug": inst.get("debug", 0),
                        "engine": inst["engine"],
                        "ins": [],
                        "outs": [],
                        "name": f"{inst['name']}.w{i}",
                        "opcode": "NoOp",
                        "sync_info": {"on_update": [], "on_wait": [w]},
                    }
                )
            si["on_wait"] = keep
        out.append(inst)
    return out


def _walk_fix(obj):
    if isinstance(obj, dict):
        for k, v in obj.items():
            if k == "instructions" and isinstance(v, list):
                obj[k] = _fix_inst_list(v)
                for inst in obj[k]:
                    _walk_fix(inst)
            else:
                _walk_fix(v)
    elif isinstance(obj, list):
        for v in obj:
            _walk_fix(v)


if not getattr(bass.Bass, "_ant_wait_split_patched", False):
    _orig_to_json_bytes = bass.Bass.to_json_bytes

    def _patched_to_json_bytes(self):
        m = orjson.loads(_orig_to_json_bytes(self))
        _walk_fix(m)
        return orjson.dumps(m)

    bass.Bass.to_json_bytes = _patched_to_json_bytes
    bass.Bass._ant_wait_split_patched = True

# ---------------------------------------------------------------------------
# Problem constants (hardcoded per task instructions)
# ---------------------------------------------------------------------------
B = 8
C = 384
H = W = 48
N = H * W          # 2304
NH = 6             # heads
D = C // NH        # 64
J3 = 3 * C         # 1152
P = 128
CT = C // P        # 3 c-tiles
NT = N // P        # 18 n-tiles
LN_EPS = 1e-5

F32 = mybir.dt.float32
BF16 = mybir.dt.bfloat16
U8 = mybir.dt.uint8
F8E4 = mybir.dt.float8e4

# fp8 Schraudolph exp on DVE.  q is pre-scaled by QSCALE = log2(e) at the
# QKV stage, so scores arrive as S'' = 11.5416*y (y = softmax logit) and
# the fp8e4 bit pattern of ~exp(y)/4 is ONE tensor_scalar:
#   bits = trunc(max(S'' + 40, 0))       (uint8 out, clamp built in)
# The 2^-2 factor matches ScalarE's exact-exp path bias EBIAS = -ln(4)
# and cancels in the softmax (denominator from the same E values).
# rms rel err ~4% (fp8 mantissa + linear-interp ripple); fp8e4 here is
# e4m3 with inf at bits>=120 -- unreachable below y = 6.9 sigma.
QSCALE = float(np.log2(np.e))          # 1.4426950408889634
S8_BIAS = 40.0                         # includes trunc-rounding calibration
SCALE8 = 0.125 / QSCALE                # ScalarE exp input scale
EBIAS = float(-np.log(4.0))            # ScalarE exp bias (exponent shift)



def build_nc(reps: int = 1) -> bass.Bass:
    nc = bass.Bass()
    x_ext = nc.dram_tensor("x", [C, N], F32, kind="ExternalInput")
    w1_ext = nc.dram_tensor("W1", [J3, C], F32, kind="ExternalInput")
    b1_ext = nc.dram_tensor("b1", [J3], F32, kind="ExternalInput")
    w2_ext = nc.dram_tensor("W2", [C, C], F32, kind="ExternalInput")
    b2_ext = nc.dram_tensor("b2", [C], F32, kind="ExternalInput")
    out_ext = nc.dram_tensor("out", [C, N], F32, kind="ExternalOutput")

    with tile.TileContext(nc) as tc:
        for _ in range(reps):
            _build_body(nc, tc, x_ext, w1_ext, b1_ext, w2_ext, b2_ext, out_ext)
    return nc


def _build_body(nc, tc, x_ext, w1_ext, b1_ext, w2_ext, b2_ext, out_ext):
    from contextlib import ExitStack

    with ExitStack() as ctx:
        singles = ctx.enter_context(tc.tile_pool(name="singles", bufs=1))

        # ---- constants -----------------------------------------------------
        ident_f32 = singles.tile([P, P], F32)
        make_identity(nc, ident_f32)
        ident_bf = singles.tile([P, P], BF16)
        make_identity(nc, ident_bf)
        eps_sb = singles.tile([P, 1], F32)
        nc.vector.memset(eps_sb, LN_EPS)
        # exp exponent-shift bias for the fp8 E path (cancels in softmax)
        eb_sb = singles.tile([P, 1], F32)
        nc.vector.memset(eb_sb, EBIAS)

        # b1 laid out partition-major per j-tile: b1_sb[p, jt] = b1[jt*128+p]
        b1_ap = b1_ext[:]
        b2_ap = b2_ext[:]
        b1_sb = singles.tile([P, J3 // P], F32)
        nc.sync.dma_start(
            out=b1_sb,
            in_=bass.AP(tensor=b1_ap.tensor, offset=b1_ap.offset,
                        ap=[[1, P], [P, J3 // P]]),
        )
        b2_sb = singles.tile([P, C // P], F32)
        nc.sync.dma_start(
            out=b2_sb,
            in_=bass.AP(tensor=b2_ap.tensor, offset=b2_ap.offset,
                        ap=[[1, P], [P, C // P]]),
        )
        # b1 slice for V, single row (broadcast via K=1 matmul later)
        b1v_f32 = singles.tile([1, C], F32)
        nc.sync.dma_start(
            out=b1v_f32,
            in_=bass.AP(tensor=b1_ap.tensor, offset=b1_ap.offset + 2 * C,
                        ap=[[1, 1], [1, C]]),
        )
        b1v_sb = singles.tile([1, C], BF16)
        nc.vector.tensor_copy(b1v_sb, b1v_f32)

        # ---- W1^T / W2^T (bf16, [c, j] layout) ----------------------------
        w1t_sb = [singles.tile([P, J3], BF16, name=f"w1t{i}") for i in range(CT)]
        w2t_sb = [singles.tile([P, C], BF16, name=f"w2t{i}") for i in range(CT)]

        with (
            tc.tile_pool(name="wrows", bufs=3) as wrows,
            tc.tile_pool(name="wpsum", bufs=4, space="PSUM") as wpsum,
        ):
            for jt in range(J3 // P):
                wr = wrows.tile([P, C], F32, tag="wrow")
                (nc.sync if jt % 2 == 0 else nc.scalar).dma_start(
                    out=wr, in_=w1_ext[jt * P:(jt + 1) * P, :])
                for ct in range(CT):
                    ps = wpsum.tile([P, P], F32, tag="wT")
                    nc.tensor.transpose(ps, wr[:, ct * P:(ct + 1) * P], ident_f32)
                    nc.any.tensor_copy(w1t_sb[ct][:, jt * P:(jt + 1) * P], ps)
            for rt in range(CT):
                wr = wrows.tile([P, C], F32, tag="wrow")
                nc.sync.dma_start(out=wr, in_=w2_ext[rt * P:(rt + 1) * P, :])
                for ct in range(CT):
                    ps = wpsum.tile([P, P], F32, tag="wT")
                    nc.tensor.transpose(ps, wr[:, ct * P:(ct + 1) * P], ident_f32)
                    nc.any.tensor_copy(w2t_sb[ct][:, rt * P:(rt + 1) * P], ps)

        # ---- persistent activations ---------------------------------------
        tn_cn = [singles.tile([P, N], BF16, name=f"tn_cn{i}") for i in range(CT)]
        qkT = [singles.tile([P, N], BF16, name=f"qkT{i}") for i in range(2 * C // P)]
        # [P, head, njt-parity, 128]: each DoubleRow k-tile is a canonical
        # 128-wide stationary (LdWeights ISA); col D = ones (softmax
        # denominator), cols D+1.. = zero pad (zeroed once by Pool)
        v8 = [singles.tile([P, NH, 2, P], F8E4, name=f"v8_{i}")
              for i in range(NT // 2)]
        for t in v8:
            nc.gpsimd.memset(t, 0.0)
            nc.gpsimd.memset(t[:, :, :, D:D + 1], 1.0)

        # ---- LayerNorm -----------------------------------------------------
        with (
            tc.tile_pool(name="xin", bufs=1) as xin,
            tc.tile_pool(name="ln", bufs=4) as ln,
            tc.tile_pool(name="lnps", bufs=5, space="PSUM") as lnps,
            tc.tile_pool(name="tn_nc_pool", bufs=4) as tn_nc_pool,
            tc.tile_pool(name="tps", bufs=3, space="PSUM") as tps,
        ):
            x_sb = [xin.tile([P, N], F32, name=f"x_sb{i}") for i in range(CT)]
            dma_engines = [nc.sync, nc.scalar, nc.gpsimd]
            for ct in range(CT):
                dma_engines[ct].dma_start(out=x_sb[ct],
                                          in_=x_ext[ct * P:(ct + 1) * P, :])

            for nt in range(NT):
                pt = lnps.tile([P, C], F32, tag="xt")      # t tile [n, c]
                for ct in range(CT):
                    nc.tensor.transpose(
                        pt[:, ct * P:(ct + 1) * P],
                        x_sb[ct][:, nt * P:(nt + 1) * P],
                        ident_f32,
                    )
                stats = ln.tile([P, nc.vector.BN_STATS_DIM], F32, tag="stats")
                nc.vector.bn_stats(out=stats, in_=pt)
                mv = ln.tile([P, nc.vector.BN_AGGR_DIM], F32, tag="mv")
                nc.vector.bn_aggr(out=mv, in_=stats)
                rstd = ln.tile([P, 1], F32, tag="rstd")
                nc.scalar.activation(
                    out=rstd, in_=mv[:, 1:2],
                    func=mybir.ActivationFunctionType.Sqrt,
                    bias=eps_sb, scale=1.0, alpha=0.0,
                )
                nc.vector.reciprocal(out=rstd, in_=rstd)
                tn = tn_nc_pool.tile([P, C], BF16, tag="tn_nc")
                nc.vector.tensor_scalar(
                    out=tn, in0=pt,
                    scalar1=mv[:, 0:1], scalar2=rstd,
                    op0=mybir.AluOpType.subtract, op1=mybir.AluOpType.mult,
                )
                # transpose t_norm back to [c, n]
                for ct in range(CT):
                    pc = tps.tile([P, P], BF16, tag="tnT")
                    nc.tensor.transpose(pc, tn[:, ct * P:(ct + 1) * P], ident_bf)
                    nc.any.tensor_copy(tn_cn[ct][:, nt * P:(nt + 1) * P], pc)

        # ---- QKV -----------------------------------------------------------
        N_SUBS = [(s, min(512, N - s)) for s in range(0, N, 512)]
        with tc.tile_pool(name="qkps", bufs=4, space="PSUM") as qkps:
            ones_row = singles.tile([1, P], BF16, name="ones_row")
            nc.vector.memset(ones_row, 1.0)
            for nt in range(NT):  # V in [n, j] layout, with ones column
                ps = qkps.tile([P, C], F32, tag="v")
                for ct in range(CT):
                    nc.tensor.matmul(
                        ps,
                        tn_cn[ct][:, nt * P:(nt + 1) * P],
                        w1t_sb[ct][:, 2 * C:3 * C],
                        start=(ct == 0), stop=False,
                    )
                # + b1v broadcast to every row via a K=1 ones-row matmul
                nc.tensor.matmul(ps, ones_row, b1v_sb[0:1, :],
                                 start=False, stop=True)
                # V lands directly in fp8 (PV runs fp8 DoubleRow for both
                # heads); ones column gives the softmax denominator for free
                nc.scalar.copy(
                    v8[nt // 2][:, :, nt % 2, 0:D],
                    ps.rearrange("p (h d) -> p h d", h=NH),
                )

            for jt in (0, 3, 1, 4, 2, 5):  # q^T/k^T j-tiles, head-pair-0 tiles first
                for s0, sl in N_SUBS:
                    ps = qkps.tile([P, 512], F32, tag="qk")
                    for ct in range(CT):
                        nc.tensor.matmul(
                            ps[:, :sl],
                            w1t_sb[ct][:, jt * P:(jt + 1) * P],
                            tn_cn[ct][:, s0:s0 + sl],
                            start=(ct == 0), stop=(ct == CT - 1),
                        )
                    if jt < CT:
                        # q tiles: fold the fp8-Schraudolph score scale in
                        nc.vector.tensor_scalar(
                            out=qkT[jt][:, s0:s0 + sl], in0=ps[:, :sl],
                            scalar1=b1_sb[:, jt:jt + 1], scalar2=QSCALE,
                            op0=mybir.AluOpType.add, op1=mybir.AluOpType.mult,
                        )
                    else:
                        nc.vector.tensor_scalar_add(
                            out=qkT[jt][:, s0:s0 + sl], in0=ps[:, :sl],
                            scalar1=b1_sb[:, jt:jt + 1],
                        )

        # ---- attention + projection, n_i-chunk outer ----------------------
        # Per 512-wide n_i chunk: all 3 head pairs run S^T -> exp -> PV,
        # normalize into o_cn[:, chunk]; then the output projection +
        # residual for that chunk issues immediately (overlaps the next
        # chunk's attention on PE/DVE while ScalarE stays exp-bound).
        o_cn = [singles.tile([P, N], BF16, name=f"o_cn{i}") for i in range(CT)]
        CHUNKS = [(s, min(512, N - s)) for s in range(0, N, 512)]
        with (
            tc.tile_pool(name="et", bufs=4) as etp,
            tc.tile_pool(name="sps", bufs=2, space="PSUM") as sps,
            tc.tile_pool(name="ops", bufs=1, space="PSUM") as ops,
            tc.tile_pool(name="rbps", bufs=1, space="PSUM") as rbps,
            tc.tile_pool(name="pps", bufs=1, space="PSUM") as pps,
            tc.tile_pool(name="nrm", bufs=4) as nrm,
            tc.tile_pool(name="outp", bufs=3) as outp,
        ):
            ones_bf = singles.tile([1, D], BF16, name="ones_bf")
            nc.vector.memset(ones_bf, 1.0)
            # Flat software-pipelined schedule over (chunk, head-pair, njt):
            # the S^T pair for step k+1 issues BEFORE the exp-dependent PV of
            # step k, so the PE fills the exp latency and ScalarE runs
            # back-to-back (HW-probed: 1647 -> 735 ns/step).
            steps = [(ci, hp, njt)
                     for ci in range(len(CHUNKS))
                     for hp in range(NH // 2)
                     for njt in range(NT)]

            def s_pair(ci, hp, njt):
                # separate PSUM tile per head so the two exp engines read
                # disjoint tiles (shared-tile reads serialize in tile deps)
                c0, cl = CHUNKS[ci]
                out = []
                for h2 in range(2):
                    ps_s = sps.tile([P, 512], F32, tag=f"S{h2}",
                                    name=f"s{h2}_{ci}_{hp}_{njt}")
                    nc.tensor.matmul(
                        ps_s[:, 0:cl],
                        qkT[NH // 2 + hp][h2 * D:(h2 + 1) * D,
                                          njt * P:(njt + 1) * P],
                        qkT[hp][h2 * D:(h2 + 1) * D, c0:c0 + cl],
                        start=True, stop=True,
                    )
                    out.append(ps_s)
                return out

            po = None
            s_pend = s_pair(*steps[0])
            pending = []   # deferred normalize/proj closures, flushed one pair later

            def make_finish(ci, hp, ou_pair):
                c0, cl = CHUNKS[ci]

                def finish():
                    # normalize: recip row -> bf16 -> K=1 ones-matmul broadcast
                    # across 64 psum partitions -> elementwise multiply.
                    for h2 in range(2):
                        ou = ou_pair[h2]
                        rrow = nrm.tile([1, 512], F32, tag="rrow",
                                        name=f"rr{ci}_{hp}_{h2}")
                        nc.vector.reciprocal(out=rrow[:, 0:cl],
                                             in_=ou[D:D + 1, 0:cl])
                        rrow_bf = nrm.tile([1, 512], BF16, tag="rrow_bf",
                                           name=f"rrb{ci}_{hp}_{h2}")
                        nc.gpsimd.tensor_copy(rrow_bf[:, 0:cl], rrow[:, 0:cl])
                        rbp = rbps.tile([P, 512], F32, tag="rb",
                                        name=f"rbp{ci}_{hp}_{h2}")
                        nc.tensor.matmul(
                            rbp[h2 * D:(h2 + 1) * D, 0:cl],
                            ones_bf,
                            rrow_bf[:, 0:cl],
                            start=True, stop=True,
                        )
                        nc.vector.tensor_tensor(
                            o_cn[hp][h2 * D:(h2 + 1) * D, c0:c0 + cl],
                            ou[0:D, 0:cl],
                            rbp[h2 * D:(h2 + 1) * D, 0:cl],
                            mybir.AluOpType.mult,
                        )
                    if hp == NH // 2 - 1:
                        # output projection + residual for this chunk
                        for rt in range(CT):
                            ps = pps.tile([P, 512], F32, tag="proj",
                                          name=f"pj{ci}_{rt}")
                            for ct in range(CT):
                                nc.tensor.matmul(
                                    ps[:, :cl],
                                    w2t_sb[ct][:, rt * P:(rt + 1) * P],
                                    o_cn[ct][:, c0:c0 + cl],
                                    start=(ct == 0), stop=(ct == CT - 1),
                                )
                            out_sb = outp.tile([P, 512], F32, tag="out",
                                               name=f"ot{ci}_{rt}")
                            nc.vector.scalar_tensor_tensor(
                                out=out_sb[:, :cl],
                                in0=ps[:, :cl],
                                scalar=b2_sb[:, rt:rt + 1],
                                in1=tn_cn[rt][:, c0:c0 + cl],
                                op0=mybir.AluOpType.add,
                                op1=mybir.AluOpType.add,
                            )
                            nc.sync.dma_start(
                                out=out_ext[rt * P:(rt + 1) * P, c0:c0 + cl],
                                in_=out_sb[:, :cl])

                return finish

            for k, (ci, hp, njt) in enumerate(steps):
                c0, cl = CHUNKS[ci]
                if njt == 0:
                    po = [ops.tile([P, 512], F32, tag=f"O{i}",
                                   name=f"po{ci}_{hp}_{i}") for i in range(2)]
                # exp split across engines, all-fp8 E: DVE computes head A
                # via the one-op fp8 Schraudolph (ScalarE takes over every
                # 8th step to balance load); ScalarE computes head B exactly
                # with the matching 2^-2 exponent shift.  Separate tiles per
                # engine -- shared-tile accesses serialize in tile deps.
                if njt % 2 == 0:
                    et_ap = etp.tile([P, 2, 512], F8E4, tag="ETA")
                    et_bp = etp.tile([P, 2, 512], F8E4, tag="ETB")
                if k % 8 == 7:
                    nc.scalar.activation(
                        out=et_ap[:, njt % 2, 0:cl], in_=s_pend[0][:, 0:cl],
                        func=mybir.ActivationFunctionType.Exp,
                        scale=SCALE8, bias=eb_sb,
                    )
                else:
                    nc.vector.tensor_scalar(
                        out=et_ap[:, njt % 2, 0:cl].bitcast(U8),
                        in0=s_pend[0][:, 0:cl],
                        scalar1=S8_BIAS, scalar2=0.0,
                        op0=mybir.AluOpType.add, op1=mybir.AluOpType.max,
                    )
                nc.scalar.activation(
                    out=et_bp[:, njt % 2, 0:cl], in_=s_pend[1][:, 0:cl],
                    func=mybir.ActivationFunctionType.Exp,
                    scale=SCALE8, bias=eb_sb,
                )
                if k + 1 < len(steps):
                    s_pend = s_pair(*steps[k + 1])
                if njt % 2 == 1:
                    # one fp8 DoubleRow matmul per head per nj-pair
                    # contracts 256 rows at 0.5 cycles/row
                    for h2, et in ((0, et_ap), (1, et_bp)):
                        nc.tensor.matmul(
                            po[h2][:, 0:cl],
                            v8[njt // 2][:, 2 * hp + h2, :, :],
                            et[:, :, 0:cl],
                            start=(njt == 1), stop=(njt == NT - 1),
                            perf_mode=mybir.MatmulPerfMode.DoubleRow,
                        )
                if njt == 4 and pending:
                    pending.pop(0)()
                if njt == NT - 1:
                    # Only stage O' out of PSUM now (frees po quickly); the
                    # PE-touching normalize/proj is deferred one head-pair so
                    # its DVE dependency chain completes off the critical path.
                    ou_pair = []
                    for h2 in range(2):
                        ou = nrm.tile([P, 512], F32, tag=f"ou{h2}",
                                      name=f"ou{ci}_{hp}_{h2}")
                        nc.scalar.copy(ou[0:D + 1, 0:cl],
                                       po[h2][0:D + 1, 0:cl])
                        ou_pair.append(ou)
                    pending.append(make_finish(ci, hp, ou_pair))
            while pending:
                pending.pop(0)()


# ---------------------------------------------------------------------------
# host-side entry point
# ---------------------------------------------------------------------------
_NC_CACHE = {}


def _get_nc(reps: int = 1):
    if reps not in _NC_CACHE:
        _NC_CACHE[reps] = build_nc(reps)
    return _NC_CACHE[reps]


def kernel(x, W1, b1, W2, b2):
    from concourse.bass_utils import run_bass_kernel_spmd

    nc = _get_nc()
    x = np.ascontiguousarray(x, dtype=np.float32)
    in_maps = [
        {
            "x": x[b].reshape(C, N),
            "W1": np.ascontiguousarray(W1, dtype=np.float32),
            "b1": np.ascontiguousarray(b1, dtype=np.float32),
            "W2": np.ascontiguousarray(W2, dtype=np.float32),
            "b2": np.ascontiguousarray(b2, dtype=np.float32),
        }
        for b in range(B)
    ]
    res = run_bass_kernel_spmd(nc, in_maps, core_ids=list(range(B)))
    out = np.stack([res.results[b]["out"] for b in range(B)], axis=0)
    return out.reshape(B, C, H, W).astype(np.float32)



# revision 41
# speedup vs baseline: 1.2201x; 1.0565x over previous
"""Trainium2 Bass kernel for nn_Attention_57543971831928.

Dense pre-LN self-attention block:
  t = x.reshape(B,C,N).T ; t_norm = LN(t)
  qkv = t_norm @ W1.T + b1 ; attention (6 heads, d=64) ; o = att_out @ W2.T + b2
  out = (o + t_norm).T.reshape(B,C,H,W)

Sharding: data-parallel over batch B=8, one batch element per NeuronCore,
no collectives.  Everything is computed in the "transposed" [c, n] /
[j, n] layout so every matmul contraction sits on the partition axis.

The kernel is softmax-exp bound: B*h*N^2/8 = 31.85M exp()s/core.  The
exp work is SPLIT across two engines, per (head, nj-tile) step of the
attention loop:
  - head B: ScalarE exact exp, output fp8e4, with an exponent-shift
    bias -ln(4) that cancels in the softmax.
  - head A: DVE one-op "Schraudolph" exp -- q is pre-scaled by log2(e)
    at the QKV stage, so trunc(max(S + 40, 0)) written as uint8 IS the
    fp8e4 bit pattern of ~exp(y)/4 (same 2^-2 factor as head B; the
    max() clamp makes negative-wrap impossible).  Every 8th step
    ScalarE takes head A too, balancing the engines.
  - PV runs fp8 MatmulPerfMode.DoubleRow (0.5 cycles/row), one matmul
    per head per nj-PAIR: E tiles are [128, 2, 512] (pair slot in free
    dim), V is [128, head, 2, 128] fp8 with a ones column for the
    softmax denominator and zero pad to the canonical 128-wide
    stationary (LdWeights ISA requires it).
  - scores stay bf16 (d=64 contraction, 1 cycle/row).

Scheduling lessons baked in (found via CoreSim timelines):
  - tile dep tracking is TILE-granular: two engines reading (or
    writing) the same tile serialize on a false dep.  Scores go to
    separate per-head PSUM tiles; E goes to separate per-engine tiles.
  - PSUM start=True zeroing is 2KB-bank granular per partition: two
    accumulation groups must not share a bank (column-split PV broke).
  - engine FIFOs execute in issue order: LN x-transposes must be
    issued before the W1^T transpose preamble or attention starts
    ~20us late; V/qk production is interleaved to pace the loop.
"""

import sys

sys.path.insert(0, "/opt/trn_rl_repo")

import numpy as np
import orjson

import concourse.bass as bass
import concourse.mybir as mybir
import concourse.tile as tile
from concourse.masks import make_identity

# ---------------------------------------------------------------------------
# Workaround: the walrus build in this container only accepts a single
# sync-wait per instruction; Tile emits instructions waiting on several
# semaphores.  Split extra waits onto preceding same-engine NoOps at BIR
# serialization time.
# ---------------------------------------------------------------------------
_SYNC_WAIT_LIMIT = 1


def _fix_inst_list(insts):
    out = []
    for inst in insts:
        si = inst.get("sync_info")
        ow = (si or {}).get("on_wait") or []
        if si is not None and len(ow) > _SYNC_WAIT_LIMIT:
            keep = ow[-_SYNC_WAIT_LIMIT:]
            extras = ow[:-_SYNC_WAIT_LIMIT]
            for i, w in enumerate(extras):
                out.append(
                    {
                        "debug": inst.get("debug", 0),
                        "engine": inst["engine"],
                        "ins": [],
                        "outs": [],
                        "name": f"{inst['name']}.w{i}",
                        "opcode": "NoOp",
                        "sync_info": {"on_update": [], "on_wait": [w]},
                    }
                )
            si["on_wait"] = keep
        out.append(inst)
    return out


def _walk_fix(obj):
    if isinstance(obj, dict):
        for k, v in obj.items():
            if k == "instructions" and isinstance(v, list):
                obj[k] = _fix_inst_list(v)
                for inst in obj[k]:
                    _walk_fix(inst)
            else:
                _walk_fix(v)
    elif isinstance(obj, list):
        for v in obj:
            _walk_fix(v)


if not getattr(bass.Bass, "_ant_wait_split_patched", False):
    _orig_to_json_bytes = bass.Bass.to_json_bytes

    def _patched_to_json_bytes(self):
        m = orjson.loads(_orig_to_json_bytes(self))
        _walk_fix(m)
        return orjson.dumps(m)

    bass.Bass.to_json_bytes = _patched_to_json_bytes
    bass.Bass._ant_wait_split_patched = True

# ---------------------------------------------------------------------------
# Problem constants (hardcoded per task instructions)
# ---------------------------------------------------------------------------
B = 8
C = 384
H = W = 48
N = H * W          # 2304
NH = 6             # heads
D = C // NH        # 64
J3 = 3 * C         # 1152
P = 128
CT = C // P        # 3 c-tiles
NT = N // P        # 18 n-tiles
LN_EPS = 1e-5

F32 = mybir.dt.float32
BF16 = mybir.dt.bfloat16
U8 = mybir.dt.uint8
F8E4 = mybir.dt.float8e4

# fp8 Schraudolph exp on DVE.  q is pre-scaled by QSCALE = log2(e) at the
# QKV stage, so scores arrive as S'' = 11.5416*y (y = softmax logit) and
# the fp8e4 bit pattern of ~exp(y)/4 is ONE tensor_scalar:
#   bits = trunc(max(S'' + 40, 0))       (uint8 out, clamp built in)
# The 2^-2 factor matches ScalarE's exact-exp path bias EBIAS = -ln(4)
# and cancels in the softmax (denominator from the same E values).
# rms rel err ~4% (fp8 mantissa + linear-interp ripple); fp8e4 here is
# e4m3 with inf at bits>=120 -- unreachable below y = 6.9 sigma.
QSCALE = float(np.log2(np.e))          # 1.4426950408889634
S8_BIAS = 40.0                         # includes trunc-rounding calibration
SCALE8 = 0.125 / QSCALE                # ScalarE exp input scale
EBIAS = float(-np.log(4.0))            # ScalarE exp bias (exponent shift)



def build_nc(reps: int = 1) -> bass.Bass:
    nc = bass.Bass()
    x_ext = nc.dram_tensor("x", [C, N], F32, kind="ExternalInput")
    w1_ext = nc.dram_tensor("W1", [J3, C], F32, kind="ExternalInput")
    b1_ext = nc.dram_tensor("b1", [J3], F32, kind="ExternalInput")
    w2_ext = nc.dram_tensor("W2", [C, C], F32, kind="ExternalInput")
    b2_ext = nc.dram_tensor("b2", [C], F32, kind="ExternalInput")
    out_ext = nc.dram_tensor("out", [C, N], F32, kind="ExternalOutput")

    with tile.TileContext(nc) as tc:
        for _ in range(reps):
            _build_body(nc, tc, x_ext, w1_ext, b1_ext, w2_ext, b2_ext, out_ext)
    return nc


def _build_body(nc, tc, x_ext, w1_ext, b1_ext, w2_ext, b2_ext, out_ext):
    from contextlib import ExitStack

    with ExitStack() as ctx:
        singles = ctx.enter_context(tc.tile_pool(name="singles", bufs=1))

        # ---- constants -----------------------------------------------------
        ident_f32 = singles.tile([P, P], F32)
        make_identity(nc, ident_f32)
        ident_bf = singles.tile([P, P], BF16)
        make_identity(nc, ident_bf)
        eps_sb = singles.tile([P, 1], F32)
        nc.vector.memset(eps_sb, LN_EPS)
        # exp exponent-shift bias for the fp8 E path (cancels in softmax)
        eb_sb = singles.tile([P, 1], F32)
        nc.vector.memset(eb_sb, EBIAS)

        # x loads issue first: the LN chain (and everything after) hangs off
        # them, while the weight loads only gate QKV.  Chunked into separate
        # tiles so LN tile 0 starts after ~1/3 of the load (tile-granular
        # deps), one DMA queue per c-tile.
        XCH = N // 3
        xin = ctx.enter_context(tc.tile_pool(name="xin", bufs=1))
        x_sb = [[xin.tile([P, XCH], F32, name=f"x_sb{i}_{j}") for j in range(3)]
                for i in range(CT)]
        for ct, eng in enumerate((nc.sync, nc.scalar, nc.gpsimd)):
            for j in range(3):
                eng.dma_start(out=x_sb[ct][j],
                              in_=x_ext[ct * P:(ct + 1) * P,
                                        j * XCH:(j + 1) * XCH])

        # b1 laid out partition-major per j-tile: b1_sb[p, jt] = b1[jt*128+p]
        b1_ap = b1_ext[:]
        b2_ap = b2_ext[:]
        b1_sb = singles.tile([P, J3 // P], F32)
        nc.sync.dma_start(
            out=b1_sb,
            in_=bass.AP(tensor=b1_ap.tensor, offset=b1_ap.offset,
                        ap=[[1, P], [P, J3 // P]]),
        )
        b2_sb = singles.tile([P, C // P], F32)
        nc.sync.dma_start(
            out=b2_sb,
            in_=bass.AP(tensor=b2_ap.tensor, offset=b2_ap.offset,
                        ap=[[1, P], [P, C // P]]),
        )
        # b1 slice for V, single row (broadcast via K=1 matmul later)
        b1v_f32 = singles.tile([1, C], F32)
        nc.sync.dma_start(
            out=b1v_f32,
            in_=bass.AP(tensor=b1_ap.tensor, offset=b1_ap.offset + 2 * C,
                        ap=[[1, 1], [1, C]]),
        )
        b1v_sb = singles.tile([1, C], BF16)
        nc.vector.tensor_copy(b1v_sb, b1v_f32)

        # ---- W1^T / W2^T (bf16, [c, j] layout) ----------------------------
        w1t_sb = [singles.tile([P, J3], BF16, name=f"w1t{i}") for i in range(CT)]
        w2t_sb = [singles.tile([P, C], BF16, name=f"w2t{i}") for i in range(CT)]

        # ---- persistent activations ---------------------------------------
        tn_cn = [singles.tile([P, N], BF16, name=f"tn_cn{i}") for i in range(CT)]
        qkT = [singles.tile([P, N], BF16, name=f"qkT{i}") for i in range(2 * C // P)]
        # [P, head, njt-parity, 128]: each DoubleRow k-tile is a canonical
        # 128-wide stationary (LdWeights ISA); col D = ones (softmax
        # denominator), cols D+1.. = zero pad (zeroed once by Pool)
        v8 = [singles.tile([P, NH, 2, P], F8E4, name=f"v8_{i}")
              for i in range(NT // 2)]
        for t in v8:
            nc.gpsimd.memset(t, 0.0)
            nc.gpsimd.memset(t[:, :, :, D:D + 1], 1.0)

        # ---- LayerNorm -----------------------------------------------------
        with (
            tc.tile_pool(name="ln", bufs=4) as ln,
            tc.tile_pool(name="lnps", bufs=5, space="PSUM") as lnps,
            tc.tile_pool(name="tn_nc_pool", bufs=4) as tn_nc_pool,
            tc.tile_pool(name="tps", bufs=3, space="PSUM") as tps,
        ):
            for nt in range(NT):
                pt = lnps.tile([P, C], F32, tag="xt")      # t tile [n, c]
                xj, xo = divmod(nt * P, XCH)
                for ct in range(CT):
                    nc.tensor.transpose(
                        pt[:, ct * P:(ct + 1) * P],
                        x_sb[ct][xj][:, xo:xo + P],
                        ident_f32,
                    )
                stats = ln.tile([P, nc.vector.BN_STATS_DIM], F32, tag="stats")
                nc.vector.bn_stats(out=stats, in_=pt)
                mv = ln.tile([P, nc.vector.BN_AGGR_DIM], F32, tag="mv")
                nc.vector.bn_aggr(out=mv, in_=stats)
                rstd = ln.tile([P, 1], F32, tag="rstd")
                nc.scalar.activation(
                    out=rstd, in_=mv[:, 1:2],
                    func=mybir.ActivationFunctionType.Sqrt,
                    bias=eps_sb, scale=1.0, alpha=0.0,
                )
                nc.vector.reciprocal(out=rstd, in_=rstd)
                # -mean*rstd, so the (heavy) normalize itself can run on
                # ScalarE as out = in*rstd + (-mean*rstd)
                nmr = ln.tile([P, 1], F32, tag="nmr")
                nc.vector.tensor_scalar(
                    out=nmr, in0=mv[:, 0:1],
                    scalar1=rstd, scalar2=-1.0,
                    op0=mybir.AluOpType.mult, op1=mybir.AluOpType.mult,
                )
                tn = tn_nc_pool.tile([P, C], BF16, tag="tn_nc")
                nc.scalar.activation(
                    out=tn, in_=pt,
                    func=mybir.ActivationFunctionType.Identity,
                    bias=nmr, scale=rstd,
                )
                # transpose t_norm back to [c, n]
                for ct in range(CT):
                    pc = tps.tile([P, P], BF16, tag="tnT")
                    nc.tensor.transpose(pc, tn[:, ct * P:(ct + 1) * P], ident_bf)
                    nc.any.tensor_copy(tn_cn[ct][:, nt * P:(nt + 1) * P], pc)

        # ---- W1^T / W2^T transposes (issued after LN so the LN-critical
        # x-transposes go first in the PE FIFO; j-tiles ordered so qk
        # head-pair-0 (0,3) and V (6-8) unblock earliest) ---------------
        with (
            tc.tile_pool(name="wrows", bufs=3) as wrows,
            tc.tile_pool(name="wpsum", bufs=4, space="PSUM") as wpsum,
        ):
            for i, jt in enumerate((0, 3, 6, 7, 8, 1, 4, 2, 5)):
                wr = wrows.tile([P, C], F32, tag="wrow")
                (nc.sync if i % 2 == 0 else nc.scalar).dma_start(
                    out=wr, in_=w1_ext[jt * P:(jt + 1) * P, :])
                for ct in range(CT):
                    ps = wpsum.tile([P, P], F32, tag="wT")
                    nc.tensor.transpose(ps, wr[:, ct * P:(ct + 1) * P], ident_f32)
                    nc.any.tensor_copy(w1t_sb[ct][:, jt * P:(jt + 1) * P], ps)
            for rt in range(CT):
                wr = wrows.tile([P, C], F32, tag="wrow")
                nc.sync.dma_start(out=wr, in_=w2_ext[rt * P:(rt + 1) * P, :])
                for ct in range(CT):
                    ps = wpsum.tile([P, P], F32, tag="wT")
                    nc.tensor.transpose(ps, wr[:, ct * P:(ct + 1) * P], ident_f32)
                    nc.any.tensor_copy(w2t_sb[ct][:, rt * P:(rt + 1) * P], ps)

        # ---- QKV -----------------------------------------------------------
        # Issue order matters: attention step 0 needs q/k head-pair-0
        # (j-tiles 0, 3) and v8 pair 0, so those go first; V production is
        # interleaved with the remaining j-tiles to keep pace with the
        # attention loop's njt sweep.
        N_SUBS = [(s, min(512, N - s)) for s in range(0, N, 512)]
        with tc.tile_pool(name="qkps", bufs=4, space="PSUM") as qkps:
            ones_row = singles.tile([1, P], BF16, name="ones_row")
            nc.vector.memset(ones_row, 1.0)

            def v_tile(nt):  # V in [n, j] fp8, with ones column
                ps = qkps.tile([P, C], F32, tag="v")
                for ct in range(CT):
                    nc.tensor.matmul(
                        ps,
                        tn_cn[ct][:, nt * P:(nt + 1) * P],
                        w1t_sb[ct][:, 2 * C:3 * C],
                        start=(ct == 0), stop=False,
                    )
                # + b1v broadcast to every row via a K=1 ones-row matmul
                nc.tensor.matmul(ps, ones_row, b1v_sb[0:1, :],
                                 start=False, stop=True)
                nc.scalar.copy(
                    v8[nt // 2][:, :, nt % 2, 0:D],
                    ps.rearrange("p (h d) -> p h d", h=NH),
                )

            def qk_chunk(jt, s0, sl):
                ps = qkps.tile([P, 512], F32, tag="qk")
                for ct in range(CT):
                    nc.tensor.matmul(
                        ps[:, :sl],
                        w1t_sb[ct][:, jt * P:(jt + 1) * P],
                        tn_cn[ct][:, s0:s0 + sl],
                        start=(ct == 0), stop=(ct == CT - 1),
                    )
                if jt < CT:
                    # q tiles: fold the fp8-Schraudolph score scale in
                    nc.vector.tensor_scalar(
                        out=qkT[jt][:, s0:s0 + sl], in0=ps[:, :sl],
                        scalar1=b1_sb[:, jt:jt + 1], scalar2=QSCALE,
                        op0=mybir.AluOpType.add, op1=mybir.AluOpType.mult,
                    )
                else:
                    nc.vector.tensor_scalar_add(
                        out=qkT[jt][:, s0:s0 + sl], in0=ps[:, :sl],
                        scalar1=b1_sb[:, jt:jt + 1],
                    )

            for jt in (0, 3):
                for s0, sl in N_SUBS:
                    qk_chunk(jt, s0, sl)
            rest = [(jt, s0, sl) for jt in (1, 4, 2, 5) for s0, sl in N_SUBS]
            for i, (jt, s0, sl) in enumerate(rest):
                if i < NT:
                    v_tile(i)
                qk_chunk(jt, s0, sl)

        # ---- attention + projection, n_i-chunk outer ----------------------
        # Per 512-wide n_i chunk: all 3 head pairs run S^T -> exp -> PV,
        # normalize into o_cn[:, chunk]; then the output projection +
        # residual for that chunk issues immediately (overlaps the next
        # chunk's attention on PE/DVE while ScalarE stays exp-bound).
        o_cn = [singles.tile([P, N], BF16, name=f"o_cn{i}") for i in range(CT)]
        CHUNKS = [(s, min(512, N - s)) for s in range(0, N, 512)]
        with (
            tc.tile_pool(name="et", bufs=4) as etp,
            tc.tile_pool(name="sps", bufs=2, space="PSUM") as sps,
            tc.tile_pool(name="ops", bufs=1, space="PSUM") as ops,
            tc.tile_pool(name="rbps", bufs=1, space="PSUM") as rbps,
            tc.tile_pool(name="pps", bufs=1, space="PSUM") as pps,
            tc.tile_pool(name="nrm", bufs=4) as nrm,
            tc.tile_pool(name="outp", bufs=3) as outp,
        ):
            ones_bf = singles.tile([1, D], BF16, name="ones_bf")
            nc.vector.memset(ones_bf, 1.0)
            # Flat software-pipelined schedule over (chunk, head-pair, njt):
            # the S^T pair for step k+1 issues BEFORE the exp-dependent PV of
            # step k, so the PE fills the exp latency and ScalarE runs
            # back-to-back (HW-probed: 1647 -> 735 ns/step).
            steps = [(ci, hp, njt)
                     for ci in range(len(CHUNKS))
                     for hp in range(NH // 2)
                     for njt in range(NT)]

            def s_pair(ci, hp, njt):
                # separate PSUM tile per head so the two exp engines read
                # disjoint tiles (shared-tile reads serialize in tile deps)
                c0, cl = CHUNKS[ci]
                out = []
                for h2 in range(2):
                    ps_s = sps.tile([P, 512], F32, tag=f"S{h2}",
                                    name=f"s{h2}_{ci}_{hp}_{njt}")
                    nc.tensor.matmul(
                        ps_s[:, 0:cl],
                        qkT[NH // 2 + hp][h2 * D:(h2 + 1) * D,
                                          njt * P:(njt + 1) * P],
                        qkT[hp][h2 * D:(h2 + 1) * D, c0:c0 + cl],
                        start=True, stop=True,
                    )
                    out.append(ps_s)
                return out

            po = None
            s_pend = s_pair(*steps[0])
            pending = []   # deferred normalize/proj closures, flushed one pair later

            def make_finish(ci, hp, ou_pair):
                c0, cl = CHUNKS[ci]

                def finish():
                    # normalize: recip row -> bf16 -> K=1 ones-matmul broadcast
                    # across 64 psum partitions -> elementwise multiply.
                    for h2 in range(2):
                        ou = ou_pair[h2]
                        rrow = nrm.tile([1, 512], F32, tag="rrow",
                                        name=f"rr{ci}_{hp}_{h2}")
                        nc.vector.reciprocal(out=rrow[:, 0:cl],
                                             in_=ou[D:D + 1, 0:cl])
                        rrow_bf = nrm.tile([1, 512], BF16, tag="rrow_bf",
                                           name=f"rrb{ci}_{hp}_{h2}")
                        nc.gpsimd.tensor_copy(rrow_bf[:, 0:cl], rrow[:, 0:cl])
                        rbp = rbps.tile([P, 512], F32, tag="rb",
                                        name=f"rbp{ci}_{hp}_{h2}")
                        nc.tensor.matmul(
                            rbp[h2 * D:(h2 + 1) * D, 0:cl],
                            ones_bf,
                            rrow_bf[:, 0:cl],
                            start=True, stop=True,
                        )
                        nc.vector.tensor_tensor(
                            o_cn[hp][h2 * D:(h2 + 1) * D, c0:c0 + cl],
                            ou[0:D, 0:cl],
                            rbp[h2 * D:(h2 + 1) * D, 0:cl],
                            mybir.AluOpType.mult,
                        )
                    if hp == NH // 2 - 1:
                        # output projection + residual for this chunk
                        for rt in range(CT):
                            ps = pps.tile([P, 512], F32, tag="proj",
                                          name=f"pj{ci}_{rt}")
                            for ct in range(CT):
                                nc.tensor.matmul(
                                    ps[:, :cl],
                                    w2t_sb[ct][:, rt * P:(rt + 1) * P],
                                    o_cn[ct][:, c0:c0 + cl],
                                    start=(ct == 0), stop=(ct == CT - 1),
                                )
                            out_sb = outp.tile([P, 512], F32, tag="out",
                                               name=f"ot{ci}_{rt}")
                            nc.vector.scalar_tensor_tensor(
                                out=out_sb[:, :cl],
                                in0=ps[:, :cl],
                                scalar=b2_sb[:, rt:rt + 1],
                                in1=tn_cn[rt][:, c0:c0 + cl],
                                op0=mybir.AluOpType.add,
                                op1=mybir.AluOpType.add,
                            )
                            nc.sync.dma_start(
                                out=out_ext[rt * P:(rt + 1) * P, c0:c0 + cl],
                                in_=out_sb[:, :cl])

                return finish

            for k, (ci, hp, njt) in enumerate(steps):
                c0, cl = CHUNKS[ci]
                if njt == 0:
                    po = [ops.tile([P, 512], F32, tag=f"O{i}",
                                   name=f"po{ci}_{hp}_{i}") for i in range(2)]
                # exp split across engines, all-fp8 E: DVE computes head A
                # via the one-op fp8 Schraudolph (ScalarE takes over every
                # 8th step to balance load); ScalarE computes head B exactly
                # with the matching 2^-2 exponent shift.  Separate tiles per
                # engine -- shared-tile accesses serialize in tile deps.
                if njt % 2 == 0:
                    et_ap = etp.tile([P, 2, 512], F8E4, tag="ETA")
                    et_bp = etp.tile([P, 2, 512], F8E4, tag="ETB")
                if k % 8 == 7:
                    nc.scalar.activation(
                        out=et_ap[:, njt % 2, 0:cl], in_=s_pend[0][:, 0:cl],
                        func=mybir.ActivationFunctionType.Exp,
                        scale=SCALE8, bias=eb_sb,
                    )
                else:
                    nc.vector.tensor_scalar(
                        out=et_ap[:, njt % 2, 0:cl].bitcast(U8),
                        in0=s_pend[0][:, 0:cl],
                        scalar1=S8_BIAS, scalar2=0.0,
                        op0=mybir.AluOpType.add, op1=mybir.AluOpType.max,
                    )
                nc.scalar.activation(
                    out=et_bp[:, njt % 2, 0:cl], in_=s_pend[1][:, 0:cl],
                    func=mybir.ActivationFunctionType.Exp,
                    scale=SCALE8, bias=eb_sb,
                )
                if k + 1 < len(steps):
                    s_pend = s_pair(*steps[k + 1])
                if njt % 2 == 1:
                    # one fp8 DoubleRow matmul per head per nj-pair
                    # contracts 256 rows at 0.5 cycles/row
                    for h2, et in ((0, et_ap), (1, et_bp)):
                        nc.tensor.matmul(
                            po[h2][:, 0:cl],
                            v8[njt // 2][:, 2 * hp + h2, :, :],
                            et[:, :, 0:cl],
                            start=(njt == 1), stop=(njt == NT - 1),
                            perf_mode=mybir.MatmulPerfMode.DoubleRow,
                        )
                if njt == 4 and pending:
                    pending.pop(0)()
                if njt == NT - 1:
                    # Only stage O' out of PSUM now (frees po quickly); the
                    # PE-touching normalize/proj is deferred one head-pair so
                    # its DVE dependency chain completes off the critical path.
                    ou_pair = []
                    for h2 in range(2):
                        ou = nrm.tile([P, 512], F32, tag=f"ou{h2}",
                                      name=f"ou{ci}_{hp}_{h2}")
                        nc.scalar.copy(ou[0:D + 1, 0:cl],
                                       po[h2][0:D + 1, 0:cl])
                        ou_pair.append(ou)
                    pending.append(make_finish(ci, hp, ou_pair))
            while pending:
                pending.pop(0)()


# ---------------------------------------------------------------------------
# host-side entry point
# ---------------------------------------------------------------------------
_NC_CACHE = {}


def _get_nc(reps: int = 1):
    if reps not in _NC_CACHE:
        _NC_CACHE[reps] = build_nc(reps)
    return _NC_CACHE[reps]


def kernel(x, W1, b1, W2, b2):
    from concourse.bass_utils import run_bass_kernel_spmd

    nc = _get_nc()
    x = np.ascontiguousarray(x, dtype=np.float32)
    in_maps = [
        {
            "x": x[b].reshape(C, N),
            "W1": np.ascontiguousarray(W1, dtype=np.float32),
            "b1": np.ascontiguousarray(b1, dtype=np.float32),
            "W2": np.ascontiguousarray(W2, dtype=np.float32),
            "b2": np.ascontiguousarray(b2, dtype=np.float32),
        }
        for b in range(B)
    ]
    res = run_bass_kernel_spmd(nc, in_maps, core_ids=list(range(B)))
    out = np.stack([res.results[b]["out"] for b in range(B)], axis=0)
    return out.reshape(B, C, H, W).astype(np.float32)



# revision 42
# speedup vs baseline: 2.1823x; 1.7886x over previous
"""Trainium2 Bass kernel for nn_Attention_57543971831928.

Dense pre-LN self-attention block:
  t = x.reshape(B,C,N).T ; t_norm = LN(t)
  qkv = t_norm @ W1.T + b1 ; attention (6 heads, d=64) ; o = att_out @ W2.T + b2
  out = (o + t_norm).T.reshape(B,C,H,W)

Sharding: data-parallel over batch B=8, one batch element per NeuronCore,
no collectives.  Everything is computed in the "transposed" [c, n] /
[j, n] layout so every matmul contraction sits on the partition axis.

The kernel is softmax-exp bound: B*h*N^2/8 = 31.85M exp()s/core.  The
exp work is SPLIT across two engines, per (head, nj-tile) step of the
attention loop:
  - head B: ScalarE exact exp, output fp8e4, with an exponent-shift
    bias -ln(4) that cancels in the softmax.
  - head A: DVE one-op "Schraudolph" exp -- q is pre-scaled by log2(e)
    at the QKV stage, so trunc(max(S + 40, 0)) written as uint8 IS the
    fp8e4 bit pattern of ~exp(y)/4 (same 2^-2 factor as head B; the
    max() clamp makes negative-wrap impossible).  Every 8th step
    ScalarE takes head A too, balancing the engines.
  - PV runs fp8 MatmulPerfMode.DoubleRow (0.5 cycles/row), one matmul
    per head per nj-PAIR: E tiles are [128, 2, 512] (pair slot in free
    dim), V is [128, head, 2, 128] fp8 with a ones column for the
    softmax denominator and zero pad to the canonical 128-wide
    stationary (LdWeights ISA requires it).
  - scores stay bf16 (d=64 contraction, 1 cycle/row).

Scheduling lessons baked in (found via CoreSim timelines):
  - tile dep tracking is TILE-granular: two engines reading (or
    writing) the same tile serialize on a false dep.  Scores go to
    separate per-head PSUM tiles; E goes to separate per-engine tiles.
  - PSUM start=True zeroing is 2KB-bank granular per partition: two
    accumulation groups must not share a bank (column-split PV broke).
  - engine FIFOs execute in issue order: LN x-transposes must be
    issued before the W1^T transpose preamble or attention starts
    ~20us late; V/qk production is interleaved to pace the loop.
"""

import sys

sys.path.insert(0, "/opt/trn_rl_repo")

import numpy as np
import orjson

import concourse.bass as bass
import concourse.mybir as mybir
import concourse.tile as tile
from concourse.masks import make_identity

# ---------------------------------------------------------------------------
# Workaround: the walrus build in this container only accepts a single
# sync-wait per instruction; Tile emits instructions waiting on several
# semaphores.  Split extra waits onto preceding same-engine NoOps at BIR
# serialization time.
# ---------------------------------------------------------------------------
_SYNC_WAIT_LIMIT = 1


_SELF_SEM_ENGINES = ("PE", "DVE", "Activation", "Pool")


def _fix_inst_list(insts):
    out = []
    for inst in insts:
        si = inst.get("sync_info")
        ow = (si or {}).get("on_wait") or []
        # Drop waits on the instruction's own engine counting sem: compute
        # engines execute their stream serially, so a wait on an EARLIER
        # same-engine instruction is satisfied by FIFO order.  This removes
        # most of the NoOps the single-wait split below would emit.
        eng = inst.get("engine")
        if si is not None and ow and eng in _SELF_SEM_ENGINES:
            pfx = eng + "_"
            ow2 = [w for w in ow
                   if not str(w.get("ant_name", "")).startswith(pfx)]
            if len(ow2) != len(ow):
                si["on_wait"] = ow = ow2
        if si is not None and len(ow) > _SYNC_WAIT_LIMIT:
            keep = ow[-_SYNC_WAIT_LIMIT:]
            extras = ow[:-_SYNC_WAIT_LIMIT]
            for i, w in enumerate(extras):
                out.append(
                    {
                        "debug": inst.get("debug", 0),
                        "engine": inst["engine"],
                        "ins": [],
                        "outs": [],
                        "name": f"{inst['name']}.w{i}",
                        "opcode": "NoOp",
                        "sync_info": {"on_update": [], "on_wait": [w]},
                    }
                )
            si["on_wait"] = keep
        out.append(inst)
    return out


def _walk_fix(obj):
    if isinstance(obj, dict):
        for k, v in obj.items():
            if k == "instructions" and isinstance(v, list):
                obj[k] = _fix_inst_list(v)
                for inst in obj[k]:
                    _walk_fix(inst)
            else:
                _walk_fix(v)
    elif isinstance(obj, list):
        for v in obj:
            _walk_fix(v)


if not getattr(bass.Bass, "_ant_wait_split_patched", False):
    _orig_to_json_bytes = bass.Bass.to_json_bytes

    def _patched_to_json_bytes(self):
        m = orjson.loads(_orig_to_json_bytes(self))
        _walk_fix(m)
        return orjson.dumps(m)

    bass.Bass.to_json_bytes = _patched_to_json_bytes
    bass.Bass._ant_wait_split_patched = True

# ---------------------------------------------------------------------------
# Problem constants (hardcoded per task instructions)
# ---------------------------------------------------------------------------
B = 8
C = 384
H = W = 48
N = H * W          # 2304
NH = 6             # heads
D = C // NH        # 64
J3 = 3 * C         # 1152
P = 128
CT = C // P        # 3 c-tiles
NT = N // P        # 18 n-tiles
LN_EPS = 1e-5

F32 = mybir.dt.float32
BF16 = mybir.dt.bfloat16
U8 = mybir.dt.uint8
F8E4 = mybir.dt.float8e4

# fp8 Schraudolph exp on DVE.  q is pre-scaled by QSCALE = log2(e) at the
# QKV stage, so scores arrive as S'' = 11.5416*y (y = softmax logit) and
# the fp8e4 bit pattern of ~exp(y)/4 is ONE tensor_scalar:
#   bits = trunc(max(S'' + 40, 0))       (uint8 out, clamp built in)
# The 2^-2 factor matches ScalarE's exact-exp path bias EBIAS = -ln(4)
# and cancels in the softmax (denominator from the same E values).
# rms rel err ~4% (fp8 mantissa + linear-interp ripple); fp8e4 here is
# e4m3 with inf at bits>=120 -- unreachable below y = 6.9 sigma.
QSCALE = float(np.log2(np.e))          # 1.4426950408889634
S8_BIAS = 40.0                         # includes trunc-rounding calibration
SCALE8 = 0.125 / QSCALE                # ScalarE exp input scale
EBIAS = float(-np.log(4.0))            # ScalarE exp bias (exponent shift)



def build_nc(reps: int = 1) -> bass.Bass:
    nc = bass.Bass()
    x_ext = nc.dram_tensor("x", [C, N], F32, kind="ExternalInput")
    w1_ext = nc.dram_tensor("W1", [J3, C], F32, kind="ExternalInput")
    b1_ext = nc.dram_tensor("b1", [J3], F32, kind="ExternalInput")
    w2_ext = nc.dram_tensor("W2", [C, C], F32, kind="ExternalInput")
    b2_ext = nc.dram_tensor("b2", [C], F32, kind="ExternalInput")
    out_ext = nc.dram_tensor("out", [C, N], F32, kind="ExternalOutput")

    with tile.TileContext(nc) as tc:
        for _ in range(reps):
            _build_body(nc, tc, x_ext, w1_ext, b1_ext, w2_ext, b2_ext, out_ext)
    return nc


def _build_body(nc, tc, x_ext, w1_ext, b1_ext, w2_ext, b2_ext, out_ext):
    from contextlib import ExitStack

    with ExitStack() as ctx:
        singles = ctx.enter_context(tc.tile_pool(name="singles", bufs=1))

        # ---- constants -----------------------------------------------------
        ident_f32 = singles.tile([P, P], F32)
        make_identity(nc, ident_f32)
        ident_bf = singles.tile([P, P], BF16)
        make_identity(nc, ident_bf)
        eps_sb = singles.tile([P, 1], F32)
        nc.vector.memset(eps_sb, LN_EPS)
        # exp exponent-shift bias for the fp8 E path (cancels in softmax)
        eb_sb = singles.tile([P, 1], F32)
        nc.vector.memset(eb_sb, EBIAS)

        # x loads issue first: the LN chain (and everything after) hangs off
        # them, while the weight loads only gate QKV.  Chunked into separate
        # tiles so LN tile 0 starts after ~1/3 of the load (tile-granular
        # deps), one DMA queue per c-tile.
        XCH = N // 3
        xin = ctx.enter_context(tc.tile_pool(name="xin", bufs=1))
        x_sb = [[xin.tile([P, XCH], F32, name=f"x_sb{i}_{j}") for j in range(3)]
                for i in range(CT)]
        for ct, eng in enumerate((nc.sync, nc.scalar, nc.gpsimd)):
            for j in range(3):
                eng.dma_start(out=x_sb[ct][j],
                              in_=x_ext[ct * P:(ct + 1) * P,
                                        j * XCH:(j + 1) * XCH])

        # b1 laid out partition-major per j-tile: b1_sb[p, jt] = b1[jt*128+p]
        b1_ap = b1_ext[:]
        b2_ap = b2_ext[:]
        b1_sb = singles.tile([P, J3 // P], F32)
        nc.sync.dma_start(
            out=b1_sb,
            in_=bass.AP(tensor=b1_ap.tensor, offset=b1_ap.offset,
                        ap=[[1, P], [P, J3 // P]]),
        )
        b2_sb = singles.tile([P, C // P], F32)
        nc.sync.dma_start(
            out=b2_sb,
            in_=bass.AP(tensor=b2_ap.tensor, offset=b2_ap.offset,
                        ap=[[1, P], [P, C // P]]),
        )
        # b1 slice for V, single row (broadcast via K=1 matmul later)
        b1v_f32 = singles.tile([1, C], F32)
        nc.sync.dma_start(
            out=b1v_f32,
            in_=bass.AP(tensor=b1_ap.tensor, offset=b1_ap.offset + 2 * C,
                        ap=[[1, 1], [1, C]]),
        )
        b1v_sb = singles.tile([1, C], BF16)
        nc.vector.tensor_copy(b1v_sb, b1v_f32)

        # ---- W1^T / W2^T (bf16, [c, j] layout) ----------------------------
        w1t_sb = [singles.tile([P, J3], BF16, name=f"w1t{i}") for i in range(CT)]
        w2t_sb = [singles.tile([P, C], BF16, name=f"w2t{i}") for i in range(CT)]

        # ---- persistent activations ---------------------------------------
        tn_cn = [singles.tile([P, N], BF16, name=f"tn_cn{i}") for i in range(CT)]
        qkT = [singles.tile([P, N], BF16, name=f"qkT{i}") for i in range(2 * C // P)]
        # [P, head, njt-parity, 128]: each DoubleRow k-tile is a canonical
        # 128-wide stationary (LdWeights ISA); col D = ones (softmax
        # denominator), cols D+1.. = zero pad (zeroed once by Pool)
        v8 = [singles.tile([P, NH, 2, P], F8E4, name=f"v8_{i}")
              for i in range(NT // 2)]
        for t in v8:
            nc.gpsimd.memset(t, 0.0)
            nc.gpsimd.memset(t[:, :, :, D:D + 1], 1.0)

        # ---- LayerNorm -----------------------------------------------------
        with (
            tc.tile_pool(name="ln", bufs=4) as ln,
            tc.tile_pool(name="lnps", bufs=5, space="PSUM") as lnps,
            tc.tile_pool(name="tn_nc_pool", bufs=4) as tn_nc_pool,
            tc.tile_pool(name="tps", bufs=3, space="PSUM") as tps,
        ):
            for nt in range(NT):
                pt = lnps.tile([P, C], F32, tag="xt")      # t tile [n, c]
                xj, xo = divmod(nt * P, XCH)
                for ct in range(CT):
                    nc.tensor.transpose(
                        pt[:, ct * P:(ct + 1) * P],
                        x_sb[ct][xj][:, xo:xo + P],
                        ident_f32,
                    )
                stats = ln.tile([P, nc.vector.BN_STATS_DIM], F32, tag="stats")
                nc.vector.bn_stats(out=stats, in_=pt)
                mv = ln.tile([P, nc.vector.BN_AGGR_DIM], F32, tag="mv")
                nc.vector.bn_aggr(out=mv, in_=stats)
                rstd = ln.tile([P, 1], F32, tag="rstd")
                nc.scalar.activation(
                    out=rstd, in_=mv[:, 1:2],
                    func=mybir.ActivationFunctionType.Sqrt,
                    bias=eps_sb, scale=1.0, alpha=0.0,
                )
                nc.vector.reciprocal(out=rstd, in_=rstd)
                # -mean*rstd, so the (heavy) normalize itself can run on
                # ScalarE as out = in*rstd + (-mean*rstd)
                nmr = ln.tile([P, 1], F32, tag="nmr")
                nc.vector.tensor_scalar(
                    out=nmr, in0=mv[:, 0:1],
                    scalar1=rstd, scalar2=-1.0,
                    op0=mybir.AluOpType.mult, op1=mybir.AluOpType.mult,
                )
                tn = tn_nc_pool.tile([P, C], BF16, tag="tn_nc")
                nc.scalar.activation(
                    out=tn, in_=pt,
                    func=mybir.ActivationFunctionType.Identity,
                    bias=nmr, scale=rstd,
                )
                # transpose t_norm back to [c, n]
                for ct in range(CT):
                    pc = tps.tile([P, P], BF16, tag="tnT")
                    nc.tensor.transpose(pc, tn[:, ct * P:(ct + 1) * P], ident_bf)
                    nc.any.tensor_copy(tn_cn[ct][:, nt * P:(nt + 1) * P], pc)

        # ---- W1^T / W2^T transposes (issued after LN so the LN-critical
        # x-transposes go first in the PE FIFO; j-tiles ordered so qk
        # head-pair-0 (0,3) and V (6-8) unblock earliest) ---------------
        with (
            tc.tile_pool(name="wrows", bufs=3) as wrows,
            tc.tile_pool(name="wpsum", bufs=4, space="PSUM") as wpsum,
        ):
            for i, jt in enumerate((0, 3, 6, 7, 8, 1, 4, 2, 5)):
                wr = wrows.tile([P, C], F32, tag="wrow")
                (nc.sync if i % 2 == 0 else nc.scalar).dma_start(
                    out=wr, in_=w1_ext[jt * P:(jt + 1) * P, :])
                for ct in range(CT):
                    ps = wpsum.tile([P, P], F32, tag="wT")
                    nc.tensor.transpose(ps, wr[:, ct * P:(ct + 1) * P], ident_f32)
                    nc.any.tensor_copy(w1t_sb[ct][:, jt * P:(jt + 1) * P], ps)
            for rt in range(CT):
                wr = wrows.tile([P, C], F32, tag="wrow")
                nc.sync.dma_start(out=wr, in_=w2_ext[rt * P:(rt + 1) * P, :])
                for ct in range(CT):
                    ps = wpsum.tile([P, P], F32, tag="wT")
                    nc.tensor.transpose(ps, wr[:, ct * P:(ct + 1) * P], ident_f32)
                    nc.any.tensor_copy(w2t_sb[ct][:, rt * P:(rt + 1) * P], ps)

        # ---- QKV -----------------------------------------------------------
        # Issue order matters: attention step 0 needs q/k head-pair-0
        # (j-tiles 0, 3) and v8 pair 0, so those go first; V production is
        # interleaved with the remaining j-tiles to keep pace with the
        # attention loop's njt sweep.
        N_SUBS = [(s, min(512, N - s)) for s in range(0, N, 512)]
        with tc.tile_pool(name="qkps", bufs=4, space="PSUM") as qkps:
            ones_row = singles.tile([1, P], BF16, name="ones_row")
            nc.vector.memset(ones_row, 1.0)

            def v_tile(nt):  # V in [n, j] fp8, with ones column
                ps = qkps.tile([P, C], F32, tag="v")
                for ct in range(CT):
                    nc.tensor.matmul(
                        ps,
                        tn_cn[ct][:, nt * P:(nt + 1) * P],
                        w1t_sb[ct][:, 2 * C:3 * C],
                        start=(ct == 0), stop=False,
                    )
                # + b1v broadcast to every row via a K=1 ones-row matmul
                nc.tensor.matmul(ps, ones_row, b1v_sb[0:1, :],
                                 start=False, stop=True)
                nc.scalar.copy(
                    v8[nt // 2][:, :, nt % 2, 0:D],
                    ps.rearrange("p (h d) -> p h d", h=NH),
                )

            def qk_chunk(jt, s0, sl):
                ps = qkps.tile([P, 512], F32, tag="qk")
                for ct in range(CT):
                    nc.tensor.matmul(
                        ps[:, :sl],
                        w1t_sb[ct][:, jt * P:(jt + 1) * P],
                        tn_cn[ct][:, s0:s0 + sl],
                        start=(ct == 0), stop=(ct == CT - 1),
                    )
                if jt < CT:
                    # q tiles: fold the fp8-Schraudolph score scale in
                    nc.vector.tensor_scalar(
                        out=qkT[jt][:, s0:s0 + sl], in0=ps[:, :sl],
                        scalar1=b1_sb[:, jt:jt + 1], scalar2=QSCALE,
                        op0=mybir.AluOpType.add, op1=mybir.AluOpType.mult,
                    )
                else:
                    nc.vector.tensor_scalar_add(
                        out=qkT[jt][:, s0:s0 + sl], in0=ps[:, :sl],
                        scalar1=b1_sb[:, jt:jt + 1],
                    )

            for jt in (0, 3):
                for s0, sl in N_SUBS:
                    qk_chunk(jt, s0, sl)
            rest = [(jt, s0, sl) for jt in (1, 4, 2, 5) for s0, sl in N_SUBS]
            for i, (jt, s0, sl) in enumerate(rest):
                if i < NT:
                    v_tile(i)
                qk_chunk(jt, s0, sl)

        # ---- attention + projection, n_i-chunk outer ----------------------
        # Per 512-wide n_i chunk: all 3 head pairs run S^T -> exp -> PV,
        # normalize into o_cn[:, chunk]; then the output projection +
        # residual for that chunk issues immediately (overlaps the next
        # chunk's attention on PE/DVE while ScalarE stays exp-bound).
        o_cn = [singles.tile([P, N], BF16, name=f"o_cn{i}") for i in range(CT)]
        CHUNKS = [(s, min(512, N - s)) for s in range(0, N, 512)]
        with (
            tc.tile_pool(name="et", bufs=4) as etp,
            tc.tile_pool(name="sps", bufs=2, space="PSUM") as sps,
            tc.tile_pool(name="ops", bufs=1, space="PSUM") as ops,
            tc.tile_pool(name="rbps", bufs=1, space="PSUM") as rbps,
            tc.tile_pool(name="pps", bufs=1, space="PSUM") as pps,
            tc.tile_pool(name="nrm", bufs=4) as nrm,
            tc.tile_pool(name="outp", bufs=3) as outp,
        ):
            ones_bf = singles.tile([1, D], BF16, name="ones_bf")
            nc.vector.memset(ones_bf, 1.0)
            # Flat software-pipelined schedule over (chunk, head-pair, njt):
            # the S^T pair for step k+1 issues BEFORE the exp-dependent PV of
            # step k, so the PE fills the exp latency and ScalarE runs
            # back-to-back (HW-probed: 1647 -> 735 ns/step).
            steps = [(ci, hp, njt)
                     for ci in range(len(CHUNKS))
                     for hp in range(NH // 2)
                     for njt in range(NT)]

            def s_pair(ci, hp, njt):
                # separate PSUM tile per head so the two exp engines read
                # disjoint tiles (shared-tile reads serialize in tile deps)
                c0, cl = CHUNKS[ci]
                out = []
                for h2 in range(2):
                    ps_s = sps.tile([P, 512], F32, tag=f"S{h2}",
                                    name=f"s{h2}_{ci}_{hp}_{njt}")
                    nc.tensor.matmul(
                        ps_s[:, 0:cl],
                        qkT[NH // 2 + hp][h2 * D:(h2 + 1) * D,
                                          njt * P:(njt + 1) * P],
                        qkT[hp][h2 * D:(h2 + 1) * D, c0:c0 + cl],
                        start=True, stop=True,
                    )
                    out.append(ps_s)
                return out

            po = None
            s_pend = s_pair(*steps[0])
            pending = []   # deferred normalize/proj closures, flushed one pair later

            def make_finish(ci, hp, ou_pair):
                c0, cl = CHUNKS[ci]

                def finish():
                    # normalize: recip row -> bf16 -> K=1 ones-matmul broadcast
                    # across 64 psum partitions -> elementwise multiply.
                    for h2 in range(2):
                        ou = ou_pair[h2]
                        rrow = nrm.tile([1, 512], F32, tag="rrow",
                                        name=f"rr{ci}_{hp}_{h2}")
                        nc.vector.reciprocal(out=rrow[:, 0:cl],
                                             in_=ou[D:D + 1, 0:cl])
                        rrow_bf = nrm.tile([1, 512], BF16, tag="rrow_bf",
                                           name=f"rrb{ci}_{hp}_{h2}")
                        nc.gpsimd.tensor_copy(rrow_bf[:, 0:cl], rrow[:, 0:cl])
                        rbp = rbps.tile([P, 512], F32, tag="rb",
                                        name=f"rbp{ci}_{hp}_{h2}")
                        nc.tensor.matmul(
                            rbp[h2 * D:(h2 + 1) * D, 0:cl],
                            ones_bf,
                            rrow_bf[:, 0:cl],
                            start=True, stop=True,
                        )
                        nc.vector.tensor_tensor(
                            o_cn[hp][h2 * D:(h2 + 1) * D, c0:c0 + cl],
                            ou[0:D, 0:cl],
                            rbp[h2 * D:(h2 + 1) * D, 0:cl],
                            mybir.AluOpType.mult,
                        )
                    if hp == NH // 2 - 1:
                        # output projection + residual for this chunk
                        for rt in range(CT):
                            ps = pps.tile([P, 512], F32, tag="proj",
                                          name=f"pj{ci}_{rt}")
                            for ct in range(CT):
                                nc.tensor.matmul(
                                    ps[:, :cl],
                                    w2t_sb[ct][:, rt * P:(rt + 1) * P],
                                    o_cn[ct][:, c0:c0 + cl],
                                    start=(ct == 0), stop=(ct == CT - 1),
                                )
                            out_sb = outp.tile([P, 512], F32, tag="out",
                                               name=f"ot{ci}_{rt}")
                            nc.vector.scalar_tensor_tensor(
                                out=out_sb[:, :cl],
                                in0=ps[:, :cl],
                                scalar=b2_sb[:, rt:rt + 1],
                                in1=tn_cn[rt][:, c0:c0 + cl],
                                op0=mybir.AluOpType.add,
                                op1=mybir.AluOpType.add,
                            )
                            nc.sync.dma_start(
                                out=out_ext[rt * P:(rt + 1) * P, c0:c0 + cl],
                                in_=out_sb[:, :cl])

                return finish

            for k, (ci, hp, njt) in enumerate(steps):
                c0, cl = CHUNKS[ci]
                if njt == 0:
                    po = [ops.tile([P, 512], F32, tag=f"O{i}",
                                   name=f"po{ci}_{hp}_{i}") for i in range(2)]
                # exp split across engines, all-fp8 E: DVE computes head A
                # via the one-op fp8 Schraudolph (ScalarE takes over every
                # 8th step to balance load); ScalarE computes head B exactly
                # with the matching 2^-2 exponent shift.  Separate tiles per
                # engine -- shared-tile accesses serialize in tile deps.
                if njt % 2 == 0:
                    et_ap = etp.tile([P, 2, 512], F8E4, tag="ETA")
                    et_bp = etp.tile([P, 2, 512], F8E4, tag="ETB")
                if k % 8 == 7:
                    nc.scalar.activation(
                        out=et_ap[:, njt % 2, 0:cl], in_=s_pend[0][:, 0:cl],
                        func=mybir.ActivationFunctionType.Exp,
                        scale=SCALE8, bias=eb_sb,
                    )
                else:
                    nc.vector.tensor_scalar(
                        out=et_ap[:, njt % 2, 0:cl].bitcast(U8),
                        in0=s_pend[0][:, 0:cl],
                        scalar1=S8_BIAS, scalar2=0.0,
                        op0=mybir.AluOpType.add, op1=mybir.AluOpType.max,
                    )
                nc.scalar.activation(
                    out=et_bp[:, njt % 2, 0:cl], in_=s_pend[1][:, 0:cl],
                    func=mybir.ActivationFunctionType.Exp,
                    scale=SCALE8, bias=eb_sb,
                )
                if k + 1 < len(steps):
                    s_pend = s_pair(*steps[k + 1])
                if njt % 2 == 1:
                    # one fp8 DoubleRow matmul per head per nj-pair
                    # contracts 256 rows at 0.5 cycles/row
                    for h2, et in ((0, et_ap), (1, et_bp)):
                        nc.tensor.matmul(
                            po[h2][:, 0:cl],
                            v8[njt // 2][:, 2 * hp + h2, :, :],
                            et[:, :, 0:cl],
                            start=(njt == 1), stop=(njt == NT - 1),
                            perf_mode=mybir.MatmulPerfMode.DoubleRow,
                        )
                if njt == 4 and pending:
                    pending.pop(0)()
                if njt == NT - 1:
                    # Only stage O' out of PSUM now (frees po quickly); the
                    # PE-touching normalize/proj is deferred one head-pair so
                    # its DVE dependency chain completes off the critical path.
                    ou_pair = []
                    for h2 in range(2):
                        ou = nrm.tile([P, 512], F32, tag=f"ou{h2}",
                                      name=f"ou{ci}_{hp}_{h2}")
                        nc.scalar.copy(ou[0:D + 1, 0:cl],
                                       po[h2][0:D + 1, 0:cl])
                        ou_pair.append(ou)
                    pending.append(make_finish(ci, hp, ou_pair))
            while pending:
                pending.pop(0)()


# ---------------------------------------------------------------------------
# host-side entry point
# ---------------------------------------------------------------------------
_NC_CACHE = {}


def _get_nc(reps: int = 1):
    if reps not in _NC_CACHE:
        _NC_CACHE[reps] = build_nc(reps)
    return _NC_CACHE[reps]


def kernel(x, W1, b1, W2, b2):
    from concourse.bass_utils import run_bass_kernel_spmd

    nc = _get_nc()
    x = np.ascontiguousarray(x, dtype=np.float32)
    in_maps = [
        {
            "x": x[b].reshape(C, N),
            "W1": np.ascontiguousarray(W1, dtype=np.float32),
            "b1": np.ascontiguousarray(b1, dtype=np.float32),
            "W2": np.ascontiguousarray(W2, dtype=np.float32),
            "b2": np.ascontiguousarray(b2, dtype=np.float32),
        }
        for b in range(B)
    ]
    res = run_bass_kernel_spmd(nc, in_maps, core_ids=list(range(B)))
    out = np.stack([res.results[b]["out"] for b in range(B)], axis=0)
    return out.reshape(B, C, H, W).astype(np.float32)

